# revision 13
# baseline (speedup 1.0000x reference)
"""Trainium2 Bass kernel v6 for MllamaTextSelfAttention (B=1, S=2048, HID=4096,
32 Q heads / 8 KV heads, HD=128, RoPE, causal mask, GQA).

Sharding: tensor-parallel over heads across 8 NeuronCores. Core c computes
Q heads [4c, 4c+4) and KV head c, plus the matching slice of the output
projection; the 8 partial outputs are summed on the host.

Measured HW facts this design is built around (microbenched in-container):
  - the PE streams 0.50 ns/column sustained (2.0 GHz P-state, not 2.4),
    with only ~5 ns/MM overhead; LDWEIGHTS is fully hidden (same-weight,
    alternating-weight and 6-weight-cycling streams all measure ~261 ns
    per N=512 bf16 matmul), so the kernel floor is set by streamed
    columns alone (~958K cols/core here).
  - fp8e4 DoubleRow measures a true 1.92x (272 ns for a 256-contraction
    N=512 MM) and is numerically correct with [p, ko, j] block layout,
    but quantizing q/k inputs fails this problem's 2e-2 absmax-rel gate
    (scores span +-17; ~5% fp8 error on scores blends softmax winners:
    measured 4.3e-2), so the kernel stays bf16 end to end.

Structure (v6):
  - A: QKV projection per 512-seq stripe; packed [q0..q3|k|v] weights and
    hidden states land as host-pre-tiled [128, 8k, 512] super-tiles in
    4+4 big contiguous DMAs per stripe, SP ring only; y writes alternate
    SP/Act rings.
  - B: per (stripe, head): scores -> exp (PSUM->SBUF, frees the score
    bank directly) -> multiplicative 0/1 bf16 causal mask on the e-tile
    (DVE) -> esum as [128,1024] pair-adds + one fold on Pool -> pso ->
    den matmul deferred behind C-chain filler -> recip -> psb broadcast
    -> normalize into ot. Diagonal k-tiles are triangularly trimmed
    (partial-width score/pso MMs, per-tile exp, [128,128] triangle mask).
  - C: output projection dripped into the B windows' PE slack as
    HALF-chains (4 MMs each) for fine-grained interleave; 6 half-chains
    reserved for each stripe's last-head tail where the den matmul
    otherwise stalls the PE; next stripe's RoPE targets are spread one
    per head-tail. Stripe 3's PSUM evictions go to DVE only (Act is
    exp-saturated there).
  - PSUM: pA 2x[128,1024] (Q-proj / score groups), pB [128,1024]
    (K+V / pso+den+psb), pC [128,1024] (out-proj; the C3 tail rotates
    pC/pA/pB).
  - loop_n variant wraps the kernel in tc.For_i for dispatch-free
    steady-state timing (SWDGE is rerouted to Act inside the loop).
"""

import math
import os
import sys

for _p in (
    "/opt/trn_rl_repo",
    "/root/.axon_site",
    "/root/.axon_site/_ro/trn_rl_repo",
    "/root/.axon_site/_ro/pypackages",
):
    if os.path.isdir(_p) and _p not in sys.path:
        sys.path.append(_p)

import numpy as np
from contextlib import ExitStack

import concourse.bass as bass
import concourse.tile as tile
from concourse import mybir
from concourse.bass_utils import run_bass_kernel_spmd

try:
    import ml_dtypes

    BF = ml_dtypes.bfloat16
except ImportError:  # pragma: no cover
    import jax.numpy as jnp

    BF = jnp.bfloat16

F32 = mybir.dt.float32
FR = mybir.dt.float32r
BF16 = mybir.dt.bfloat16
ACTF = mybir.ActivationFunctionType

B, S, HID = 1, 2048, 4096
NH, NKV, HD = 32, 8, 128
NCORES = 8
QH = NH // NCORES          # 4 q heads per core
SS = 512                   # sequence stripe
NQS = S // SS              # 4 stripes
NKT = S // 128             # 16 k tiles over full seq
KH = HID // 128            # 32 hidden-dim contraction tiles
KG = 8                     # k-tiles per DMA super-tile
NG = KH // KG              # 4 super-tile groups
WPK = QH * 128 + 256       # packed weight row width per k-tile (768)
NEG = -1e9


def _split_multi_waits(nc: bass.Bass):
    """Walrus in this container encodes at most ONE sync-wait command per
    instruction. Hoist extra waits onto injected same-engine NoOps placed
    immediately before the instruction; engines are in-order so the
    semantics are unchanged."""
    n = 0
    for fn in nc.m.functions:
        for bb in fn.blocks:
            out = []
            for inst in bb.instructions:
                si = inst.sync_info
                if si is not None and si.on_wait and len(si.on_wait) > 1:
                    waits = list(si.on_wait)
                    for w in waits[:-1]:
                        n += 1
                        nop = mybir.InstNoOp(name=f"I-swait-{n}", ins=[], outs=[])
                        nop.engine = inst.engine
                        nop.sync_info = mybir.SyncInfo(on_wait=[w], on_update=[])
                        out.append(nop)
                    si.on_wait = [waits[-1]]
                out.append(inst)
            bb.instructions[:] = out
    return nc


_BUILD_CACHE = {}


def _build(causal: bool, split_waits: bool = True, loop_n=None, ablate_b=False) -> bass.Bass:
    key = (causal, split_waits, loop_n, ablate_b)
    if key in _BUILD_CACHE:
        return _BUILD_CACHE[key]

    nc = bass.Bass()
    # host-pre-tiled layouts: hT8[p, n, k, s] = h[k*128+p, n*512+s]
    hT8 = nc.dram_tensor("hT8", [128, NQS * KH * SS], BF16, kind="ExternalInput")
    # wqkv8[p, k, j] = packed-weights[k*128+p, j], j < 768
    wqkv8 = nc.dram_tensor("wqkv8", [128, KH * WPK], BF16, kind="ExternalInput")
    woT = nc.dram_tensor("woT", [QH * HD, HID], BF16, kind="ExternalInput")
    cosT = nc.dram_tensor("cosT", [HD, S], BF16, kind="ExternalInput")
    sinT = nc.dram_tensor("sinT", [HD, S], BF16, kind="ExternalInput")
    if causal:
        m01d = nc.dram_tensor("m01d", [128, 4 * SS], BF16, kind="ExternalInput")
    else:
        maskT = nc.dram_tensor("maskT", [S, S], BF16, kind="ExternalInput")
    y = nc.dram_tensor("y", [S, HID], BF16, kind="ExternalOutput")

    with tile.TileContext(nc) as tc, ExitStack() as ctx:
        if loop_n is not None:
            # device-side repeat loop for dispatch-amortized timing; SWDGE
            # (gpsimd) DMA inside For_i fails this walrus' codegen, so the
            # loop variant issues the background prefetches from Act instead
            ctx.enter_context(tc.For_i(0, loop_n, 1))
        bg = nc.scalar if loop_n is not None else nc.gpsimd
        wp = ctx.enter_context(tc.tile_pool(name="wp", bufs=1))
        hp = ctx.enter_context(tc.tile_pool(name="hp", bufs=2))
        vp = ctx.enter_context(tc.tile_pool(name="vp", bufs=2))
        ep = ctx.enter_context(tc.tile_pool(name="ep", bufs=2))
        rp = ctx.enter_context(tc.tile_pool(name="rp", bufs=2))
        pp = ctx.enter_context(tc.tile_pool(name="pp", bufs=1, space="PSUM"))

        # ---- persistent SBUF ----
        # packed per-k projection weights: [q0|q1|q2|q3|k|v] along free dim,
        # 8 k-tiles per super-tile
        wq8 = [wp.tile([128, KG * WPK], BF16, name=f"wq8_{g}") for g in range(NG)]
        wo_sb = wp.tile([128, QH * HID], BF16)
        cos_sb = wp.tile([128, S], BF16)
        sin_sb = wp.tile([128, S], BF16)
        qT = wp.tile([128, QH * S], BF16)    # [d, (stripe, head, s)]
        kT = wp.tile([128, S], BF16)         # [d, s]
        v_sb = wp.tile([128, S], BF16)       # [s-in-tile, (t, d)]
        ot = wp.tile([128, QH * S], BF16)    # [d, (stripe, head, s)] normalized O^T
        ones_f = wp.tile([128, 128], F32)
        nc.vector.memset(ones_f[:], 1.0)
        ones = wp.tile([128, 128], FR, name="ones_fr")
        nc.vector.tensor_copy(ones[:], ones_f[:])
        if causal:
            m01_sb = wp.tile([128, 4 * SS], BF16)
        else:
            mrow = wp.tile([128, NKT * SS], BF16)   # mask row-block for one stripe

        # ---- initial DMAs (SP ring: weights + hidden super-tiles only) ----
        ht0 = []
        for g in range(NG):
            nc.sync.dma_start(wq8[g][:], wqkv8[:, g * KG * WPK : (g + 1) * KG * WPK])
            ht = hp.tile([128, KG * SS], BF16, tag="ht8")
            nc.sync.dma_start(ht[:], hT8[:, g * KG * SS : (g + 1) * KG * SS])
            ht0.append(ht)
        # background prefetch on the software-DGE queue (Act ring in loop)
        bg.dma_start(cos_sb[:], cosT[:, :])
        bg.dma_start(sin_sb[:], sinT[:, :])
        if causal:
            bg.dma_start(m01_sb[:], m01d[:, :])
        for hh in range(QH):
            bg.dma_start(
                wo_sb[:, hh * HID : (hh + 1) * HID],
                woT[hh * 128 : (hh + 1) * 128, :],
            )

        # ---------------- phase emitters ----------------

        def emit_A(n, interleave=None):
            """QKV projection stripe n + V transpose. interleave is an
            iterator of closures (C-phase chains) to emit between k-tiles."""
            psq01 = pp.tile([128, 1024], F32, tag="pA", bufs=2)
            psq23 = pp.tile([128, 1024], F32, tag="pA", bufs=2)
            pskv = pp.tile([128, 1024], F32, tag="pB")
            for g in range(NG):
                if n == 0:
                    ht = ht0[g]
                else:
                    ht = hp.tile([128, KG * SS], BF16, tag="ht8")
                    nc.sync.dma_start(
                        ht[:], hT8[:, (n * KH + g * KG) * SS : (n * KH + (g + 1) * KG) * SS]
                    )
                for k8 in range(KG):
                    k = g * KG + k8
                    w0 = k8 * WPK
                    hts = ht[:, k8 * SS : (k8 + 1) * SS]
                    st, sp = (k == 0), (k == KH - 1)
                    nc.tensor.matmul(psq01[:, 0:512], wq8[g][:, w0 : w0 + 128], hts, start=st, stop=sp)
                    nc.tensor.matmul(psq01[:, 512:1024], wq8[g][:, w0 + 128 : w0 + 256], hts, start=st, stop=sp)
                    nc.tensor.matmul(psq23[:, 0:512], wq8[g][:, w0 + 256 : w0 + 384], hts, start=st, stop=sp)
                    nc.tensor.matmul(psq23[:, 512:1024], wq8[g][:, w0 + 384 : w0 + 512], hts, start=st, stop=sp)
                    nc.tensor.matmul(pskv[:, 0:512], wq8[g][:, w0 + 512 : w0 + 640], hts, start=st, stop=sp)
                    nc.tensor.matmul(pskv[:, 512:1024], wq8[g][:, w0 + 640 : w0 + 768], hts, start=st, stop=sp)
                    if interleave is not None and k % 2 == 1:
                        chain = next(interleave, None)
                        if chain is not None:
                            chain()
            base = n * (QH * SS)
            nc.vector.tensor_copy(qT[:, base : base + 1024], psq01[:])
            nc.vector.tensor_copy(qT[:, base + 1024 : base + 2048], psq23[:])
            nc.scalar.copy(kT[:, n * SS : (n + 1) * SS], pskv[:, 0:512])
            vstage = vp.tile([128, SS], BF16, tag="vst")
            nc.scalar.copy(vstage[:], pskv[:, 512:1024])

            # V transpose via DMA xbar (SP hardware queue), [128,128] blocks
            for j in range(4):
                t = 4 * n + j
                nc.sync.dma_start_transpose(
                    v_sb[:, t * 128 : (t + 1) * 128],
                    vstage[:, j * 128 : (j + 1) * 128],
                )

        def emit_rope(n):
            """RoPE on the 4 q-head stripes + the k stripe of stripe n (DVE,
            in place, bf16). Emitted late (after B(n-1)) so the DVE work never
            queues ahead of e-tile mask muls."""
            base = n * (QH * SS)
            cs = cos_sb[:, n * SS : (n + 1) * SS]
            sn = sin_sb[:, n * SS : (n + 1) * SS]
            targets = [qT[:, base + m * SS : base + (m + 1) * SS] for m in range(QH)]
            targets.append(kT[:, n * SS : (n + 1) * SS])
            for src in targets:
                rot = rp.tile([128, SS], BF16, tag="rot")
                tmp = rp.tile([128, SS], BF16, tag="tmp")
                nc.vector.tensor_scalar_mul(rot[0:64, :], src[64:128, :], -1.0)
                nc.vector.tensor_copy(rot[64:128, :], src[0:64, :])
                nc.vector.tensor_mul(tmp[:], src, cs)
                nc.vector.tensor_mul(rot[:], rot[:], sn)
                nc.vector.tensor_add(src, tmp[:], rot[:])

        def _pair_order(qs):
            nkt = 4 * qs + 4 if causal else NKT
            allp = list(range(0, nkt, 2))
            if causal:
                # plain pairs first: the first pair then initialises esum at
                # full width, and the diagonal pairs' mask muls get maximum
                # slack before pso consumes them (they are consumed last)
                diag = [t for t in allp if t >= 4 * qs]
                rest = [t for t in allp if t < 4 * qs]
                return rest + diag
            return allp

        def _tile_off(qs, t):
            """first needed q-column of k-tile t in stripe qs (causal trim)"""
            if not causal or qs == 0 or t < 4 * qs:
                return 0
            return (t - 4 * qs) * 128

        def score_pair_closures(qs, h, box, tag="pA", pbufs=2):
            """Per-pair closures computing scores + exp (+ causal mask) +
            esum for head h of stripe qs. The first closure allocates the
            e/esum tiles into `box`; running all closures == one head's
            scores. Closures can be dripped into a preceding A phase (with
            tag='pC', whose banks are free there) so the B window opens with
            e-tiles already materialised."""
            pairs = _pair_order(qs)
            cls = []
            for gi, t0 in enumerate(pairs):
                cls.append(lambda gi=gi, t0=t0: _score_pair(
                    qs, h, box, gi, t0, len(pairs), tag, pbufs))
            return cls

        def _score_pair(qs, h, box, gi, t0, npairs, tag, pbufs):
            if gi == 0:
                box["e"] = ep.tile([128, NKT * SS], BF16, tag="e", name="e")
                box["es2"] = ep.tile([128, 1024], FR, tag="es2", bufs=1, name="es2")
                box["esf"] = ep.tile([128, SS], FR, tag="esf", name="esf")
            e, esum2, esumf = box["e"], box["es2"], box["esf"]
            qsl = qT[:, qs * (QH * SS) + h * SS : qs * (QH * SS) + (h + 1) * SS]
            if True:
                pss = pp.tile([128, 1024], F32, tag=tag, bufs=pbufs, name="pss")
                offs = [_tile_off(qs, t0), _tile_off(qs, t0 + 1)]
                for half in range(2):
                    t = t0 + half
                    o = offs[half]
                    nc.tensor.matmul(
                        pss[:, half * 512 + o : half * 512 + 512],
                        kT[:, t * 128 : (t + 1) * 128],
                        qsl[:, o:SS],
                        start=True,
                        stop=True,
                    )
                if not causal:
                    nc.vector.tensor_add(
                        pss[:], pss[:], mrow[:, t0 * SS : (t0 + 2) * SS]
                    )
                if offs == [0, 0]:
                    # full-width pair: one exp over both tiles
                    eg = e[:, t0 * SS : (t0 + 2) * SS]
                    nc.scalar.activation(eg, pss[:], ACTF.Exp)
                    if causal and t0 >= 4 * qs:
                        j = t0 - 4 * qs
                        nc.vector.tensor_mul(
                            eg, eg, m01_sb[:, j * SS : (j + 2) * SS]
                        )
                    if gi == 0:
                        nc.gpsimd.tensor_copy(esum2[:], eg)
                    else:
                        nc.gpsimd.tensor_add(esum2[:], esum2[:], eg)
                else:
                    # trimmed diagonal pair: per-tile exp on the live range,
                    # [128,128] triangle mask mul, range-wise esum adds
                    for half in range(2):
                        t = t0 + half
                        o = offs[half]
                        j = t - 4 * qs
                        eg = e[:, t * SS + o : (t + 1) * SS]
                        nc.scalar.activation(
                            eg, pss[:, half * 512 + o : half * 512 + 512], ACTF.Exp
                        )
                        nc.vector.tensor_mul(
                            e[:, t * SS + o : t * SS + o + 128],
                            e[:, t * SS + o : t * SS + o + 128],
                            m01_sb[:, j * SS + o : j * SS + o + 128],
                        )
                        nc.gpsimd.tensor_add(
                            esum2[:, half * 512 + o : half * 512 + 512],
                            esum2[:, half * 512 + o : half * 512 + 512],
                            eg,
                        )
            if gi == npairs - 1:
                nc.gpsimd.tensor_add(
                    esumf[:], esum2[:, 0:512], esum2[:, 512:1024]
                )

        def emit_scores(qs, h):
            box = {}
            for cl in score_pair_closures(qs, h, box):
                cl()
            return box["e"], box["esf"]

        def emit_pso(qs, h, e, interleave=None):
            """pso accumulation for head h of stripe qs. Returns psB."""
            nkt = 4 * qs + 4 if causal else NKT
            psB = pp.tile([128, 1024], F32, tag="pB")
            # consume e in the same order scores produced it, and drip C-chain
            # filler between sub-chains so the PE never waits at the exp rate
            ts = [t0 + half for t0 in _pair_order(qs) for half in range(2)]
            for i, t in enumerate(ts):
                o = _tile_off(qs, t)
                nc.tensor.matmul(
                    psB[:, o:512],
                    v_sb[:, t * 128 : (t + 1) * 128],
                    e[:, t * SS + o : (t + 1) * SS],
                    start=(i == 0),
                    stop=(i == nkt - 1),
                )
                if interleave is not None and i % 2 == 1 and i != nkt - 1:
                    chain = next(interleave, None)
                    if chain is not None:
                        chain()
            return psB

        def emit_den(qs, h, psB, esumf):
            """den matmul + recip; returns the psb+normalize finisher to emit
            a chunk later (hides recip/hop latency from the PE)."""
            nc.tensor.matmul(
                psB[0:1, 512:1024], ones[:, 0:1], esumf[:], start=True, stop=True
            )
            rec = rp.tile([1, SS], FR, tag="rec")
            with nc.allow_low_precision(reason="fp32r recip feeds matmul"):
                nc.vector.reciprocal(rec[:], psB[0:1, 512:1024])

            def fin():
                nc.tensor.matmul(
                    psB[:, 512:1024], ones[0:1, 0:128], rec[:],
                    start=True, stop=True,
                )
                od = ot[:, qs * (QH * SS) + h * SS : qs * (QH * SS) + (h + 1) * SS]
                nc.scalar.copy(od, psB[:, 0:512])
                nc.vector.tensor_mul(od, od, psB[:, 512:1024])

            return fin

        def emit_B(qs, chains=None, nchunk=4, pre2=None):
            if not causal:
                for t in range(NKT):
                    nc.sync.dma_start(
                        mrow[:, t * SS : (t + 1) * SS],
                        maskT[t * 128 : (t + 1) * 128, qs * SS : (qs + 1) * SS],
                    )
            # reserve 3 C chains for the last head's tail, where the den
            # matmul otherwise stalls the PE on the Pool esum chain
            if chains:
                main_it = iter(chains[:-6])
                res_it = iter(chains[-6:])
            else:
                main_it = res_it = None

            def chunk(it, k=nchunk):
                if it is not None:
                    for _ in range(k):
                        chain = next(it, None)
                        if chain is not None:
                            chain()

            # spread next stripe's rope targets through this stripe so the
            # DVE work never bursts ahead of the e-tile mask muls
            rope_cl = []
            if qs + 1 < NQS:
                base = (qs + 1) * (QH * SS)
                cs = cos_sb[:, (qs + 1) * SS : (qs + 2) * SS]
                sn = sin_sb[:, (qs + 1) * SS : (qs + 2) * SS]
                targets = [
                    qT[:, base + m * SS : base + (m + 1) * SS] for m in range(QH)
                ]
                targets.append(kT[:, (qs + 1) * SS : (qs + 2) * SS])
                def mk(src):
                    def go():
                        rot = rp.tile([128, SS], BF16, tag="rot")
                        tmp = rp.tile([128, SS], BF16, tag="tmp")
                        nc.vector.tensor_scalar_mul(rot[0:64, :], src[64:128, :], -1.0)
                        nc.vector.tensor_copy(rot[64:128, :], src[0:64, :])
                        nc.vector.tensor_mul(tmp[:], src, cs)
                        nc.vector.tensor_mul(rot[:], rot[:], sn)
                        nc.vector.tensor_add(src, tmp[:], rot[:])
                    return go
                rope_cl = [mk(t) for t in targets]

            def tail(h, e, esumf, last=False):
                it = res_it if last else main_it
                psB = emit_pso(qs, h, e, interleave=main_it)
                chunk(it)
                fin = emit_den(qs, h, psB, esumf)
                chunk(it, 1)
                fin()
                if rope_cl:
                    rope_cl.pop(0)()

            if pre2 is not None:
                # heads 0-1 were pre-scored inside the previous A phase:
                # open the window with dense pso work instead of waiting on
                # the first exp chain
                b0, b1 = pre2
                tail(0, b0["e"], b0["esf"])
                b2 = {}
                for cl in score_pair_closures(qs, 2, b2):
                    cl()
                tail(1, b1["e"], b1["esf"])
                b3 = {}
                for cl in score_pair_closures(qs, 3, b3):
                    cl()
                tail(2, b2["e"], b2["esf"])
                tail(3, b3["e"], b3["esf"], last=True)
            else:
                prev = None
                for h in range(QH):
                    cur = emit_scores(qs, h)
                    if prev is not None:
                        tail(h - 1, *prev)
                    prev = cur
                tail(QH - 1, *prev, last=True)
            for it in (main_it, res_it):
                if it is not None:
                    for chain in it:
                        chain()
            for cl in rope_cl:
                cl()

        def pre_score2(qs):
            """Closures scoring heads 0-1 of stripe qs on the pC bank, for
            dripping into A(qs+1)'s dense stream (causal path only: the
            non-causal mrow buffer is loaded per-stripe by emit_B)."""
            boxes = ({}, {})
            cls = []
            for h in range(2):
                cls += score_pair_closures(qs, h, boxes[h], tag="pC", pbufs=None)
            return boxes, cls

        def C_chains(qs, tags=("pC",), copy_eng=("vector",)):
            """Output projection for stripe qs as a list of closures.
            copy_eng: engines cycled for the PSUM->SBUF staging copy (the
            Pool engine cannot read PSUM on real hardware). All y writes ride
            the Act HWDGE ring, keeping the SP ring free for loads."""
            chains = []
            for st in range(4):
                for nnp in range(4):
                    i = st * 4 + nnp
                    box = {}
                    def part1(st=st, nnp=nnp, i=i, box=box):
                        tag = tags[i % len(tags)]
                        psy = pp.tile(
                            [128, 1024], F32, tag=tag, bufs=2 if tag == "pA" else None
                        )
                        box["psy"] = psy
                        nn = nnp * 2
                        for hh in range(QH):
                            nc.tensor.matmul(
                                psy[:, 0:512],
                                ot[:, qs * (QH * SS) + hh * SS + st * 128 : qs * (QH * SS) + hh * SS + (st + 1) * 128],
                                wo_sb[:, hh * HID + nn * 512 : hh * HID + (nn + 1) * 512],
                                start=(hh == 0),
                                stop=(hh == QH - 1),
                            )
                    def part2(st=st, nnp=nnp, i=i, box=box):
                        psy = box["psy"]
                        nn = nnp * 2 + 1
                        for hh in range(QH):
                            nc.tensor.matmul(
                                psy[:, 512:1024],
                                ot[:, qs * (QH * SS) + hh * SS + st * 128 : qs * (QH * SS) + hh * SS + (st + 1) * 128],
                                wo_sb[:, hh * HID + nn * 512 : hh * HID + (nn + 1) * 512],
                                start=(hh == 0),
                                stop=(hh == QH - 1),
                            )
                        yt = vp.tile([128, 1024], BF16, tag="yt", bufs=4)
                        eng = copy_eng[i % len(copy_eng)]
                        if eng == "scalar":
                            nc.scalar.copy(yt[:], psy[:])
                        else:
                            nc.vector.tensor_copy(yt[:], psy[:])
                        row = (qs * 4 + st) * 128
                        deng = nc.sync if i % 2 else nc.scalar
                        deng.dma_start(
                            y[row : row + 128, nnp * 1024 : (nnp + 1) * 1024], yt[:]
                        )
                    chains.append(part1)
                    chains.append(part2)
            return chains

        # ---------------- pipeline ----------------
        if ablate_b:
            # timing diagnostic: A + C phases only (attention removed); ot is
            # filled with a constant so C consumes defined data
            nc.vector.memset(ot[:], 0.01)
            emit_A(0)
            emit_A(1)
            c0 = C_chains(0, copy_eng=("scalar",) * 8 + ("vector",) * 8)
            emit_A(2, interleave=iter(c0[:8]))
            for chain in c0[8:]:
                chain()
            c1 = C_chains(1, copy_eng=("scalar",) * 8 + ("vector",) * 8)
            emit_A(3, interleave=iter(c1[:8]))
            for chain in c1[8:]:
                chain()
            for chain in C_chains(2, copy_eng=("vector",)):
                chain()
            for chain in C_chains(3, tags=("pC", "pA", "pB"), copy_eng=("scalar", "vector")):
                chain()
        else:
            # All C chains go into the B windows: the dense A phases have no
            # PE slack to fill, while B's cross-engine exp chains leave the
            # PE short of work.
            emit_A(0)
            emit_rope(0)
            if causal:
                p0, c0p = pre_score2(0)
                emit_A(1, interleave=iter(c0p))
                emit_B(0, pre2=p0)
                p1, c1p = pre_score2(1)
                emit_A(2, interleave=iter(c1p))
                emit_B(1, chains=C_chains(0, copy_eng=("vector", "scalar")), pre2=p1)
                p2, c2p = pre_score2(2)
                emit_A(3, interleave=iter(c2p))
                emit_B(2, chains=C_chains(1, copy_eng=("vector", "scalar")), pre2=p2)
            else:
                emit_A(1)
                emit_B(0)
                emit_A(2)
                emit_B(1, chains=C_chains(0, copy_eng=("vector", "scalar")))
                emit_A(3)
                emit_B(2, chains=C_chains(1, copy_eng=("vector", "scalar")))
            emit_B(3, chains=C_chains(2, copy_eng=("vector",)))
            for chain in C_chains(3, tags=("pC", "pA", "pB"), copy_eng=("scalar", "vector")):
                chain()

    if split_waits:
        _split_multi_waits(nc)
    _BUILD_CACHE[key] = nc
    return nc


def _causal_mask_ref() -> np.ndarray:
    return np.triu(np.full((S, S), NEG, np.float32), k=1)


def _diag_mask01_tiles() -> np.ndarray:
    p = np.arange(128, dtype=np.int64)[:, None]
    f = np.arange(SS, dtype=np.int64)[None, :]
    cols = [
        np.where(128 * j + p > f, np.float32(0.0), np.float32(1.0)) for j in range(4)
    ]
    return np.ascontiguousarray(np.concatenate(cols, axis=1).astype(np.float32))


def make_in_maps(hidden_states, attention_mask, cos, sin, wq, wk, wv, wo):
    """Host-side sharding/preprocessing. Returns (causal, in_maps)."""
    h = np.ascontiguousarray(np.asarray(hidden_states, dtype=np.float32)[0])
    m2 = np.ascontiguousarray(np.asarray(attention_mask, dtype=np.float32)[0, 0])
    wq = np.asarray(wq, dtype=np.float32)
    wk = np.asarray(wk, dtype=np.float32)
    wv = np.asarray(wv, dtype=np.float32)
    wo = np.asarray(wo, dtype=np.float32)

    causal = bool(np.array_equal(m2, _causal_mask_ref()))
    hT = np.ascontiguousarray(h.T).astype(BF)          # [HID, S]
    # hT8[p, n, k, s] = hT[k*128+p, n*512+s]
    hT8 = np.ascontiguousarray(
        hT.reshape(KH, 128, NQS, SS).transpose(1, 2, 0, 3).reshape(128, NQS * KH * SS)
    )
    cosT = np.ascontiguousarray(np.asarray(cos, dtype=np.float32)[0].T).astype(BF)
    sinT = np.ascontiguousarray(np.asarray(sin, dtype=np.float32)[0].T).astype(BF)
    sc = np.float32(1.0 / math.sqrt(HD))
    if causal:
        m01 = _diag_mask01_tiles().astype(BF)
    else:
        mT = np.ascontiguousarray(m2.T).astype(BF)

    in_maps = []
    for c in range(NCORES):
        wqkv = np.concatenate(
            [
                (wq[c * QH * HD : (c + 1) * QH * HD] * sc).T,
                wk[c * HD : (c + 1) * HD].T,
                wv[c * HD : (c + 1) * HD].T,
            ],
            axis=1,
        ).astype(BF)                                    # [HID, 768]
        # wqkv8[p, k, j] = wqkv[k*128+p, j]
        wqkv8 = np.ascontiguousarray(
            wqkv.reshape(KH, 128, WPK).transpose(1, 0, 2).reshape(128, KH * WPK)
        )
        im = {
            "hT8": hT8,
            "cosT": cosT,
            "sinT": sinT,
            "wqkv8": wqkv8,
            "woT": np.ascontiguousarray(wo[:, c * QH * HD : (c + 1) * QH * HD].T).astype(BF),
        }
        if causal:
            im["m01d"] = m01
        else:
            im["maskT"] = mT
        in_maps.append(im)
    return causal, in_maps


def kernel(hidden_states, attention_mask, cos, sin, wq, wk, wv, wo):
    causal, in_maps = make_in_maps(
        hidden_states, attention_mask, cos, sin, wq, wk, wv, wo
    )
    nc = _build(causal)
    res = run_bass_kernel_spmd(nc, in_maps, list(range(NCORES)))
    out = np.zeros((S, HID), np.float64)
    for c in range(NCORES):
        out += res.results[c]["y"].astype(np.float64)
    return out.reshape(B, S, HID).astype(np.float32)
